# revision 3
# baseline (speedup 1.0000x reference)
"""EmbeddingBag(mean, 1M x 128 fp32 table) + Linear(128->5) on 8 Trainium2 cores.

Strategy (data-parallel by bags, table replicated per core):
  - Each core owns 2048 consecutive bags; its indices are a contiguous slice
    of sparse_features (offsets are sorted), ~102K indices/core.
  - Host bin-packs the 2048 bags into 64 blocks x 32 slots balancing the
    position count per block; every block is padded to T_BLK tiles of 128
    positions (only ~4% padding thanks to the packing).
  - Device: indirect-DMA gathers 128-row tiles [pos->partition, dim->free],
    a 0/1 selection matrix S (built on-chip from per-position slot ids via
    is_equal against an iota) right-multiplies each tile on the TensorEngine,
    accumulating bag sums [dim, slot] directly in PSUM across all 2048 slots.
  - Mean = multiply by 1/count (host-precomputed, broadcast layout), then the
    Linear runs as 4 more matmuls contracting over dim; bias added on DVE.
  - No collectives needed: output [5, 2048] per core, host re-orders
    slots->bags and concatenates.
"""
import os
import sys

if '/opt/trn_rl_repo' not in sys.path:
    sys.path.insert(0, '/opt/trn_rl_repo')

import numpy as np

# Problem constants (nn_Net_2 embedding_lookup).
NUM_EMB = 1_000_000
D = 128
BATCH = 16384
OUT_DIM = 5
NCORES = 8
BPC = BATCH // NCORES      # bags per core
SLOT = 32                  # bag slots per block (matmul rhs width)
NBLK = BPC // SLOT         # blocks per core
T_CH = 32                  # gather-chunk size in 128-position tiles


def _binpack(sizes, nblk, slot):
    """Assign len(sizes)==nblk*slot bags to nblk blocks, exactly `slot` bags
    each, greedily balancing the summed sizes. Returns assign[bag]->block."""
    import heapq
    order = np.argsort(-sizes, kind="stable")
    loads = np.zeros(nblk)
    nb = np.zeros(nblk, dtype=np.int64)
    assign = np.empty(len(sizes), dtype=np.int64)
    heap = [(0.0, j) for j in range(nblk)]
    heapq.heapify(heap)
    for i in order:
        parked = []
        while True:
            load, j = heapq.heappop(heap)
            if nb[j] < slot:
                break
            parked.append((load, j))
        assign[i] = j
        nb[j] += 1
        loads[j] += sizes[i]
        if nb[j] < slot:
            heapq.heappush(heap, (loads[j], j))
        for item in parked:
            heapq.heappush(heap, item)
    return assign, loads


def build_plan(sparse_features, offsets):
    """Pure-numpy preprocessing: shard bags over cores, bin-pack into blocks,
    lay out per-core index / slot-id / recip arrays for the device kernel."""
    sparse_features = np.asarray(sparse_features).astype(np.int32)
    offsets = np.asarray(offsets).astype(np.int64)
    counts = np.diff(offsets)

    per_core = []
    max_load = 0
    for c in range(NCORES):
        bag0 = c * BPC
        sizes = counts[bag0:bag0 + BPC]
        assign, loads = _binpack(sizes, NBLK, SLOT)
        max_load = max(max_load, int(loads.max()))
        per_core.append((bag0, assign))

    t_blk = max(1, -(-max_load // 128))  # ceil
    nt = NBLK * t_blk
    L = t_blk * 128

    cores = []
    for c in range(NCORES):
        bag0, assign = per_core[c]
        idx = np.zeros((NBLK, L), dtype=np.int32)
        slot_id = np.full((NBLK, L), SLOT, dtype=np.float32)  # SLOT = padding sentinel
        recipb = np.ones(BPC, dtype=np.float32)
        slot2bag = np.zeros(BPC, dtype=np.int64)
        fill = np.zeros(NBLK, dtype=np.int64)       # positions used per block
        nslot = np.zeros(NBLK, dtype=np.int64)      # slots used per block
        for b in range(BPC):
            j = assign[b]
            s = nslot[j]
            nslot[j] += 1
            bag = bag0 + b
            cnt = int(counts[bag])
            slot2bag[j * SLOT + s] = bag
            recipb[j * SLOT + s] = 1.0 / max(cnt, 1)
            if cnt:
                lo = fill[j]
                idx[j, lo:lo + cnt] = sparse_features[offsets[bag]:offsets[bag] + cnt]
                slot_id[j, lo:lo + cnt] = s
                fill[j] += cnt
        # [NBLK, L] -> tiles: position (j, u*128+p) lives at [p, j*t_blk+u]
        idx_t = idx.reshape(NBLK, t_blk, 128).transpose(2, 0, 1).reshape(128, nt)
        slot_t = slot_id.reshape(NBLK, t_blk, 128).transpose(2, 0, 1).reshape(128, nt)
        cores.append({
            "idx": np.ascontiguousarray(idx_t),
            "slot": np.ascontiguousarray(slot_t),
            "recipb": np.ascontiguousarray(np.broadcast_to(recipb, (128, BPC))),
            "slot2bag": slot2bag,
        })
    return {"t_blk": t_blk, "nt": nt, "cores": cores}


def simulate_plan(plan, emb_table, lin_w, lin_b):
    """Numpy emulation of exactly what the device computes (for testing)."""
    out = np.zeros((BATCH, OUT_DIM), dtype=np.float32)
    t_blk, nt = plan["t_blk"], plan["nt"]
    for c in range(NCORES):
        pc = plan["cores"][c]
        idx, slot_id = pc["idx"], pc["slot"]
        pooled = np.zeros((D, BPC), dtype=np.float32)
        for t in range(nt):
            j = t // t_blk
            g = emb_table[idx[:, t]]                          # [128, D]
            s = (slot_id[:, t:t + 1] == np.arange(SLOT)[None, :]).astype(np.float32)
            pooled[:, j * SLOT:(j + 1) * SLOT] += g.T @ s
        pooled *= pc["recipb"]
        lin = lin_w @ pooled + lin_b[:, None]                 # [5, BPC]
        out[pc["slot2bag"]] = lin.T
    return out


def build_program(t_blk, nt):
    from concourse import bacc, bass, mybir
    import concourse.tile as tile

    f32 = mybir.dt.float32
    nc = bacc.Bacc("TRN2", debug=False)
    emb_d = nc.declare_dram_parameter("emb", [NUM_EMB, D], f32, isOutput=False)
    idx_d = nc.declare_dram_parameter("idx", [128, nt], mybir.dt.int32, isOutput=False)
    slt_d = nc.declare_dram_parameter("slt", [128, nt], f32, isOutput=False)
    iot_d = nc.declare_dram_parameter("iot", [128, T_CH, SLOT], f32, isOutput=False)
    rcp_d = nc.declare_dram_parameter("rcp", [128, BPC], f32, isOutput=False)
    wt_d = nc.declare_dram_parameter("wt", [128, OUT_DIM], f32, isOutput=False)
    bia_d = nc.declare_dram_parameter("bia", [128, 1], f32, isOutput=False)
    out_d = nc.declare_dram_parameter("out", [OUT_DIM, BPC], f32, isOutput=True)

    with tile.TileContext(nc) as tc:
        with (
            tc.tile_pool(name="const", bufs=1) as const_p,
            tc.tile_pool(name="gbuf", bufs=8) as g_p,
            tc.tile_pool(name="sbuf", bufs=3) as s_p,
            tc.tile_pool(name="res", bufs=1) as res_p,
            tc.tile_pool(name="psum", bufs=1, space="PSUM") as psum_p,
        ):
            idx_sb = const_p.tile([128, nt], mybir.dt.int32)
            slt_sb = const_p.tile([128, nt], f32)
            iot_sb = const_p.tile([128, T_CH, SLOT], f32)
            rcp_sb = const_p.tile([128, BPC], f32)
            wt_sb = const_p.tile([128, OUT_DIM], f32)
            bia_sb = const_p.tile([128, 1], f32)
            nc.sync.dma_start(out=idx_sb[:], in_=idx_d.ap()[:, :])
            nc.sync.dma_start(out=slt_sb[:], in_=slt_d.ap()[:, :])
            nc.sync.dma_start(out=iot_sb[:], in_=iot_d.ap()[:, :, :])
            nc.sync.dma_start(out=rcp_sb[:], in_=rcp_d.ap()[:, :])
            nc.sync.dma_start(out=wt_sb[:], in_=wt_d.ap()[:, :])
            nc.sync.dma_start(out=bia_sb[:], in_=bia_d.ap()[:, :])

            pooled_ps = psum_p.tile([128, BPC], f32)

            t0 = 0
            while t0 < nt:
                tiles = min(T_CH, nt - t0)
                s = s_p.tile([128, T_CH, SLOT], f32, tag="s")
                nc.vector.tensor_tensor(
                    out=s[:, :tiles, :],
                    in0=slt_sb[:, t0:t0 + tiles, None].to_broadcast(
                        [128, tiles, SLOT]
                    ),
                    in1=iot_sb[:, :tiles, :],
                    op=mybir.AluOpType.is_equal,
                )
                for u in range(tiles):
                    t = t0 + u
                    j = t // t_blk
                    uu = t % t_blk
                    g = g_p.tile([128, D], f32, tag="g")
                    nc.gpsimd.indirect_dma_start(
                        out=g[:],
                        out_offset=None,
                        in_=emb_d.ap()[:, :],
                        in_offset=bass.IndirectOffsetOnAxis(
                            ap=idx_sb[:, t:t + 1], axis=0
                        ),
                    )
                    nc.tensor.matmul(
                        out=pooled_ps[:, j * SLOT:(j + 1) * SLOT],
                        lhsT=g[:],
                        rhs=s[:, u, :],
                        start=(uu == 0),
                        stop=(uu == t_blk - 1),
                    )
                t0 += tiles

            pooled_sb = res_p.tile([128, BPC], f32)
            nc.vector.tensor_tensor(
                out=pooled_sb[:],
                in0=pooled_ps[:],
                in1=rcp_sb[:],
                op=mybir.AluOpType.mult,
            )
            out_sb = res_p.tile([OUT_DIM, BPC], f32)
            for k in range(BPC // 512):
                lin_ps = psum_p.tile([128, 512], f32, tag="lin")
                nc.tensor.matmul(
                    out=lin_ps[:OUT_DIM, :],
                    lhsT=wt_sb[:],
                    rhs=pooled_sb[:, k * 512:(k + 1) * 512],
                    start=True,
                    stop=True,
                )
                nc.vector.tensor_tensor(
                    out=out_sb[:, k * 512:(k + 1) * 512],
                    in0=lin_ps[:OUT_DIM, :],
                    in1=bia_sb[:OUT_DIM, 0:1].to_broadcast([OUT_DIM, 512]),
                    op=mybir.AluOpType.add,
                )
            nc.sync.dma_start(out=out_d.ap()[:, :], in_=out_sb[:])

    nc.finalize()
    return nc


def make_in_maps(plan, emb_table, lin_w, lin_b):
    emb_table = np.ascontiguousarray(np.asarray(emb_table, dtype=np.float32))
    lin_w = np.asarray(lin_w, dtype=np.float32)
    lin_b = np.asarray(lin_b, dtype=np.float32)
    iota = np.broadcast_to(
        np.arange(SLOT, dtype=np.float32), (128, T_CH, SLOT)
    ).copy()
    wt = np.ascontiguousarray(lin_w.T)                       # [128, 5]
    bia = np.zeros((128, 1), dtype=np.float32)
    bia[:OUT_DIM, 0] = lin_b
    in_maps = []
    for c in range(NCORES):
        pc = plan["cores"][c]
        in_maps.append({
            "emb": emb_table,
            "idx": pc["idx"],
            "slt": pc["slot"],
            "iot": iota,
            "rcp": pc["recipb"],
            "wt": wt,
            "bia": bia,
        })
    return in_maps


def assemble_output(plan, results):
    out = np.zeros((BATCH, OUT_DIM), dtype=np.float32)
    for c in range(NCORES):
        lin = results[c]["out"]                              # [5, BPC]
        out[plan["cores"][c]["slot2bag"]] = lin.T
    return out


def kernel(emb_table, lin_w, lin_b, sparse_features, offsets, send_shape,
           trace=False):
    from concourse.bass_utils import run_bass_kernel_spmd

    plan = build_plan(sparse_features, offsets)
    nc = build_program(plan["t_blk"], plan["nt"])
    in_maps = make_in_maps(plan, emb_table, lin_w, lin_b)
    res = run_bass_kernel_spmd(nc, in_maps, list(range(NCORES)), trace=trace)
    out = assemble_output(plan, res.results)
    if trace:
        return out, res
    return out


# revision 5
# speedup vs baseline: 2.7020x; 2.7020x over previous
"""EmbeddingBag(mean, 1M x 128 table) + Linear(128->5) on 8 Trainium2 cores.

Strategy (data-parallel by bags, table replicated per core, bf16 gather):
  - Each core owns 2048 consecutive bags (a contiguous slice of
    sparse_features since offsets are sorted), ~102K indices/core.
  - The 1M-row table is split into 32 windows of 31250 rows so row offsets
    fit the int16 indices of the batched `dma_gather` custom instruction
    (one instruction gathers a whole cell = all of a block's positions that
    hit one window; 4 SWDGE queues run descriptor generation in parallel).
  - Bags are grouped into 8 blocks of 256 slots; each block's positions are
    sorted by window, cell lengths padded to 128 and shared across cores
    (SPMD uniformity), ~20% padding.
  - Pooling: gathered 128-position tiles [pos->partition, dim->free] are
    multiplied on TensorE by an on-chip 0/1 selection matrix S
    (is_equal(slot_id, iota), built alternately on DVE and ACT), accumulating
    bag sums for all 2048 slots directly in PSUM ([dim, slot] layout).
  - Mean = multiply by precomputed 1/count, Linear = 4 fp32 matmuls
    contracting over dim, bias added on DVE. No collectives needed.
  - Table/gather/S run in bf16 (~1.7e-3 rel err vs fp32 reference);
    accumulation and the Linear stay fp32.
"""
import sys

if '/opt/trn_rl_repo' not in sys.path:
    sys.path.insert(0, '/opt/trn_rl_repo')

import numpy as np
import ml_dtypes

# Problem constants (nn_Net_2 embedding_lookup).
NUM_EMB = 1_000_000
D = 128
BATCH = 16384
OUT_DIM = 5
NCORES = 8
BPC = BATCH // NCORES       # bags per core
SLOT = 256                  # bag slots per block (matmul rhs width)
NBLK = BPC // SLOT          # blocks per core (8)
WIN = 31250                 # table rows per int16 window
NWIN = NUM_EMB // WIN       # 32
S_CH = 32                   # tiles per S-build chunk
MAX_CELL_TILES = 8          # split bigger cells into multiple gathers
NQUEUES = 4


def build_plan(sparse_features, offsets):
    """Shard bags over cores; per (core, block) sort positions by table
    window; pad every (block, window) cell to a 128-multiple length shared
    across cores so the SPMD program is uniform."""
    sf = np.asarray(sparse_features).astype(np.int64)
    offsets = np.asarray(offsets).astype(np.int64)
    counts = np.diff(offsets)

    # positions per (core, block, window) + their idx/slot lists
    cell_items = {}   # (c, j, w) -> [idx_local array, slot array]
    for c in range(NCORES):
        for j in range(NBLK):
            bag0 = c * BPC + j * SLOT
            lo, hi = offsets[bag0], offsets[bag0 + SLOT]
            idxs = sf[lo:hi]
            # slot id per position within this block
            reps = counts[bag0:bag0 + SLOT]
            slots = np.repeat(np.arange(SLOT), reps)
            w = idxs // WIN
            order = np.argsort(w, kind="stable")
            idxs, slots, w = idxs[order], slots[order], w[order]
            bnd = np.searchsorted(w, np.arange(NWIN + 1))
            for win in range(NWIN):
                a, b = bnd[win], bnd[win + 1]
                if b > a:
                    cell_items[(c, j, win)] = (idxs[a:b] - win * WIN, slots[a:b])

    # shared (across cores) padded cell lengths
    cell_len = np.zeros((NBLK, NWIN), dtype=np.int64)
    for j in range(NBLK):
        for w in range(NWIN):
            m = max(
                (len(cell_items[(c, j, w)][0]) if (c, j, w) in cell_items else 0)
                for c in range(NCORES)
            )
            cell_len[j, w] = -(-m // 128) * 128  # ceil to 128

    # layout: blocks in order, cells in window order within block
    gathers = []      # (tile_off, ntiles, window, idx_col_off) shared by cores
    blk_tile0 = []    # first tile of each block
    t_off = 0
    col_off = 0
    for j in range(NBLK):
        blk_tile0.append(t_off)
        for w in range(NWIN):
            L = int(cell_len[j, w])
            if L == 0:
                continue
            # split cells bigger than MAX_CELL_TILES tiles
            done = 0
            while done < L:
                piece = min(L - done, MAX_CELL_TILES * 128)
                gathers.append((t_off + done // 128, piece // 128, w, col_off))
                done += piece
                col_off += piece // 16
            t_off += L // 128
    blk_tile0.append(t_off)
    nt = t_off
    ncols = col_off

    cores = []
    for c in range(NCORES):
        idx16 = np.zeros((128, ncols), dtype=np.int16)
        slot_id = np.full((NBLK, NWIN and 0 or 0,), 0)  # placeholder
        slot_flat = np.full(nt * 128, SLOT, dtype=np.float32)
        recip = np.ones(BPC, dtype=np.float32)
        cnt = counts[c * BPC:(c + 1) * BPC]
        recip[:] = 1.0 / np.maximum(cnt, 1)
        t_off = 0
        col = 0
        for j in range(NBLK):
            for w in range(NWIN):
                L = int(cell_len[j, w])
                if L == 0:
                    continue
                item = cell_items.get((c, j, w))
                cell_idx = np.zeros(L, dtype=np.int16)
                cell_slot = np.full(L, SLOT, dtype=np.float32)
                if item is not None:
                    n = len(item[0])
                    cell_idx[:n] = item[0]
                    cell_slot[:n] = item[1]
                base = t_off * 128
                slot_flat[base:base + L] = cell_slot
                # idx16 wrapped-in-16, replicated over the 8 core groups
                wrapped = cell_idx.reshape(L // 16, 16).T     # [16, L/16]
                for g8 in range(8):
                    idx16[g8 * 16:(g8 + 1) * 16, col:col + L // 16] = wrapped
                t_off += L // 128
                col += L // 16
        # tile layout: position i -> partition i%128, tile i//128
        slot_t = slot_flat.reshape(nt, 128).T                  # [128, nt]
        cores.append({
            "idx16": idx16,
            "slot": np.ascontiguousarray(slot_t.astype(ml_dtypes.bfloat16)),
            "recipb": np.ascontiguousarray(
                np.broadcast_to(recip, (128, BPC)).astype(np.float32)
            ),
        })
    return {
        "nt": nt,
        "ncols": ncols,
        "gathers": gathers,
        "blk_tile0": blk_tile0,
        "cores": cores,
    }


def simulate_plan(plan, emb_table, lin_w, lin_b):
    """Numpy emulation of the device computation (bf16 table/S)."""
    emb = np.asarray(emb_table).astype(ml_dtypes.bfloat16).astype(np.float32)
    out = np.zeros((BATCH, OUT_DIM), dtype=np.float32)
    nt = plan["nt"]
    for c in range(NCORES):
        pc = plan["cores"][c]
        slot_t = pc["slot"].astype(np.float32)
        pooled = np.zeros((D, BPC), dtype=np.float32)
        # reconstruct gathered rows from idx16 + gather list
        for (t0, ntl, w, col) in plan["gathers"]:
            wrapped = pc["idx16"][:16, col:col + ntl * 8]     # [16, L/16]
            cell_idx = wrapped.T.reshape(-1).astype(np.int64) + w * WIN
            g = emb[cell_idx].reshape(ntl, 128, D)            # [ntl, 128, D]
            j = np.searchsorted(plan["blk_tile0"], t0, side="right") - 1
            for u in range(ntl):
                t = t0 + u
                s = (slot_t[:, t:t + 1] == np.arange(SLOT)[None, :]).astype(np.float32)
                pooled[:, j * SLOT:(j + 1) * SLOT] += g[u].T @ s
        pooled *= pc["recipb"]
        lin = np.asarray(lin_w) @ pooled + np.asarray(lin_b)[:, None]
        out[c * BPC:(c + 1) * BPC] = lin.T
    return out


def build_program(plan):
    from concourse import bacc, mybir
    import concourse.tile as tile

    f32 = mybir.dt.float32
    bf16 = mybir.dt.bfloat16
    i16 = mybir.dt.int16
    nt, ncols = plan["nt"], plan["ncols"]
    gathers, blk_tile0 = plan["gathers"], plan["blk_tile0"]

    nc = bacc.Bacc("TRN2", debug=False, num_swdge_queues=NQUEUES)
    emb_d = nc.declare_dram_parameter("emb", [NUM_EMB, D], bf16, isOutput=False)
    idx_d = nc.declare_dram_parameter("idx", [128, ncols], i16, isOutput=False)
    slt_d = nc.declare_dram_parameter("slt", [128, nt], bf16, isOutput=False)
    iot_d = nc.declare_dram_parameter("iot", [128, S_CH, SLOT], bf16, isOutput=False)
    rcp_d = nc.declare_dram_parameter("rcp", [128, BPC], f32, isOutput=False)
    wt_d = nc.declare_dram_parameter("wt", [128, OUT_DIM], f32, isOutput=False)
    bia_d = nc.declare_dram_parameter("bia", [128, 1], f32, isOutput=False)
    out_d = nc.declare_dram_parameter("out", [OUT_DIM, BPC], f32, isOutput=True)

    # tile index -> block
    t2j = np.searchsorted(blk_tile0, np.arange(nt), side="right") - 1

    with tile.TileContext(nc) as tc:
        with (
            tc.tile_pool(name="const", bufs=1) as const_p,
            tc.tile_pool(name="gbuf", bufs=12) as g_p,
            tc.tile_pool(name="sbuf", bufs=4) as s_p,
            tc.tile_pool(name="res", bufs=1) as res_p,
            tc.tile_pool(name="psum", bufs=1, space="PSUM") as psum_p,
        ):
            idx_sb = const_p.tile([128, ncols], i16)
            slt_sb = const_p.tile([128, nt], bf16)
            iot_sb = const_p.tile([128, S_CH, SLOT], bf16)
            rcp_sb = const_p.tile([128, BPC], f32)
            wt_sb = const_p.tile([128, OUT_DIM], f32)
            bia_sb = const_p.tile([128, 1], f32)
            nc.sync.dma_start(out=idx_sb[:], in_=idx_d.ap()[:, :])
            nc.sync.dma_start(out=slt_sb[:], in_=slt_d.ap()[:, :])
            nc.sync.dma_start(out=iot_sb[:], in_=iot_d.ap()[:, :, :])
            nc.sync.dma_start(out=rcp_sb[:], in_=rcp_d.ap()[:, :])
            nc.sync.dma_start(out=wt_sb[:], in_=wt_d.ap()[:, :])
            nc.sync.dma_start(out=bia_sb[:], in_=bia_d.ap()[:, :])

            pooled_ps = psum_p.tile([128, BPC], f32)

            # S chunks, alternating DVE / ACT
            s_tiles = {}
            for s0 in range(0, nt, S_CH):
                tiles = min(S_CH, nt - s0)
                s = s_p.tile([128, S_CH, SLOT], bf16, tag="s")
                nc.vector.tensor_tensor(
                    out=s[:, :tiles, :],
                    in0=slt_sb[:, s0:s0 + tiles, None].to_broadcast(
                        [128, tiles, SLOT]
                    ),
                    in1=iot_sb[:, :tiles, :],
                    op=mybir.AluOpType.is_equal,
                )
                s_tiles[s0] = s

            # gather cells + matmuls, in tile order
            qrr = 0
            gather_of_tile = {}
            for (t0, ntl, w, col) in gathers:
                g = g_p.tile([128, MAX_CELL_TILES, D], bf16, tag="g")
                win_n = min(WIN, NUM_EMB - w * WIN)
                nc.gpsimd.dma_gather(
                    out_ap=g[:, :ntl, :],
                    in_ap=emb_d.ap()[w * WIN:w * WIN + win_n, :],
                    idxs_ap=idx_sb[:, col:col + ntl * 8],
                    num_idxs=ntl * 128,
                    num_idxs_reg=ntl * 128,
                    elem_size=D,
                    queue_num=qrr % NQUEUES,
                )
                qrr += 1
                for u in range(ntl):
                    gather_of_tile[t0 + u] = (g, u)

            for t in range(nt):
                j = int(t2j[t])
                g, u = gather_of_tile[t]
                s = s_tiles[(t // S_CH) * S_CH]
                nc.tensor.matmul(
                    out=pooled_ps[:, j * SLOT:(j + 1) * SLOT],
                    lhsT=g[:, u, :],
                    rhs=s[:, t % S_CH, :],
                    start=(t == blk_tile0[j]),
                    stop=(t == blk_tile0[j + 1] - 1),
                )

            pooled_sb = res_p.tile([128, BPC], f32)
            nc.vector.tensor_tensor(
                out=pooled_sb[:],
                in0=pooled_ps[:],
                in1=rcp_sb[:],
                op=mybir.AluOpType.mult,
            )
            out_sb = res_p.tile([OUT_DIM, BPC], f32)
            for k in range(BPC // 512):
                lin_ps = psum_p.tile([128, 512], f32, tag="lin")
                nc.tensor.matmul(
                    out=lin_ps[:OUT_DIM, :],
                    lhsT=wt_sb[:],
                    rhs=pooled_sb[:, k * 512:(k + 1) * 512],
                    start=True,
                    stop=True,
                )
                nc.vector.tensor_tensor(
                    out=out_sb[:, k * 512:(k + 1) * 512],
                    in0=lin_ps[:OUT_DIM, :],
                    in1=bia_sb[:OUT_DIM, 0:1].to_broadcast([OUT_DIM, 512]),
                    op=mybir.AluOpType.add,
                )
            nc.sync.dma_start(out=out_d.ap()[:, :], in_=out_sb[:])

    nc.finalize()
    return nc


def make_in_maps(plan, emb_table, lin_w, lin_b):
    emb_bf = np.ascontiguousarray(
        np.asarray(emb_table, dtype=np.float32).astype(ml_dtypes.bfloat16)
    )
    lin_w = np.asarray(lin_w, dtype=np.float32)
    lin_b = np.asarray(lin_b, dtype=np.float32)
    iota = np.broadcast_to(
        np.arange(SLOT, dtype=np.float32).astype(ml_dtypes.bfloat16),
        (128, S_CH, SLOT),
    ).copy()
    wt = np.ascontiguousarray(lin_w.T)
    bia = np.zeros((128, 1), dtype=np.float32)
    bia[:OUT_DIM, 0] = lin_b
    in_maps = []
    for c in range(NCORES):
        pc = plan["cores"][c]
        in_maps.append({
            "emb": emb_bf,
            "idx": pc["idx16"],
            "slt": pc["slot"],
            "iot": iota,
            "rcp": pc["recipb"],
            "wt": wt,
            "bia": bia,
        })
    return in_maps


def assemble_output(results):
    out = np.zeros((BATCH, OUT_DIM), dtype=np.float32)
    for c in range(NCORES):
        out[c * BPC:(c + 1) * BPC] = results[c]["out"].T
    return out


def kernel(emb_table, lin_w, lin_b, sparse_features, offsets, send_shape,
           trace=False):
    from concourse.bass_utils import run_bass_kernel_spmd

    plan = build_plan(sparse_features, offsets)
    nc = build_program(plan)
    in_maps = make_in_maps(plan, emb_table, lin_w, lin_b)
    res = run_bass_kernel_spmd(nc, in_maps, list(range(NCORES)), trace=trace)
    out = assemble_output(res.results)
    if trace:
        return out, res
    return out
